# revision 22
# baseline (speedup 1.0000x reference)
"""Butterfly (nn_Butterfly) forward as a single dense matmul on 8 TRN2 cores.

The reference butterfly network is linear in x: h starts as (x, 0) complex
pairs, every perm/diag factor is a real-linear map with coefficients that
depend only on (perm_logit, abcd), and the output takes the real part and
adds b.  So forward(x) == x @ M + b where M = forward(I_1024) with b=0.
M is built on the host from the ~16KB params (cheap, exact), then the
device kernel is a data-parallel [2048,1024] @ [1024,1024] matmul per core.

Structure (v3):
  - x is pre-transposed on the host during sharding, so the contraction
    dim lands on SBUF partitions straight off the DMA (no PE transposes).
  - bf16 operands: measured on HW, bf16 matmuls stream at the 215ns/MM
    roofline with 97ns (FWL) weight loads, while f32r runs ~15% slower
    with 200ns weight loads.  x is DMA'd as f32 on the sync HWDGE ring
    into a staging tile and cast f32->bf16 by the otherwise-idle ACT
    engine (the SWDGE cast-on-DMA path starts ~5us late and trickles).
  - DMA schedule: M chunks + bias + stores on the sync (SP) ring,
    x chunks on the scalar (ACT) ring.  The two rings drain concurrently
    (SDMA round-robins per packet), so each ramp kt's (M, x) chunk pair
    lands in ~1.4us < the 1.73us of matmuls it feeds - the PE never
    starves after the first chunk.  kt0's chunks are split finer so the
    first matmul issues as early as possible.
  - PSUM: 8 accumulator banks; ramp runs 4 btiles kt-major against the
    arriving chunks, steady state runs one btile at a time.
  - stores are per-512-column half, so the tail after the last matmul is
    one DVE add + one 256KB store.
"""

import numpy as np

N = 1024
B_FULL = 16384
N_CORES = 8
B_CORE = B_FULL // N_CORES  # 2048
N_KT = N // 128  # 8 contraction tiles
N_GROUPS = 4  # x column groups per core
GCOLS = B_CORE // N_GROUPS  # 512 batch columns per group
N_BT = B_CORE // 128  # 16 output row tiles

# "bf16": bf16 matmuls, x cast on ACT (fastest measured)
# "f32r": both operands f32r (exact fp32; ~15% slower PE stream on HW)
MM_MODE = "bf16"


# ---------------------------------------------------------------------------
# Host side: collapse the butterfly network to a single matrix
# ---------------------------------------------------------------------------

def _abcd_offsets(n):
    offs = []
    off = 0
    m = n
    while m >= 2:
        offs.append((m, off))
        off += 2 * m
        m //= 2
    return offs, off


def _np_forward(x, perm_logit, abcd, b):
    """Float64 numpy port of reference._forward (op-for-op)."""
    x = np.asarray(x, np.float64)
    perm_logit = np.asarray(perm_logit, np.float64)
    abcd = np.asarray(abcd, np.float64)
    b = np.asarray(b, np.float64)
    n = x.shape[-1]
    Bn = x.shape[0]
    offs, _ = _abcd_offsets(n)
    h = np.stack([x, np.zeros_like(x)], axis=-1)
    perm_sizes = [m for (m, _) in offs if m >= 4]
    for d in range(perm_logit.shape[0]):
        p = 1.0 / (1.0 + np.exp(-perm_logit[d]))
        for m in reversed(perm_sizes):
            h = h.reshape(Bn, n // m, m, 2)
            eo = np.concatenate([h[:, :, 0::2], h[:, :, 1::2]], axis=2)
            h = (1 - p[0]) * h + p[0] * eo
            h1, h2 = h[:, :, : m // 2], h[:, :, m // 2 :]
            h1 = (1 - p[1]) * h1 + p[1] * h1[:, :, ::-1]
            h2 = (1 - p[2]) * h2 + p[2] * h2[:, :, ::-1]
            h = np.concatenate([h1, h2], axis=2).reshape(Bn, n, 2)
        for (m, off) in reversed(offs):
            ABCD = abcd[d, off : off + 2 * m].reshape(2, 2, m // 2, 2)
            hv = h.reshape(Bn, n // m, 2, m // 2, 2)
            xr, xi = hv[..., 0], hv[..., 1]
            Ar, Ai = ABCD[..., 0], ABCD[..., 1]
            yr = np.einsum("ijk,bnjk->bnik", Ar, xr) - np.einsum(
                "ijk,bnjk->bnik", Ai, xi
            )
            yi = np.einsum("ijk,bnjk->bnik", Ar, xi) + np.einsum(
                "ijk,bnjk->bnik", Ai, xr
            )
            h = np.stack([yr, yi], axis=-1).reshape(Bn, n, 2)
    return b + h[..., 0]


def _build_matrix(perm_logit, abcd):
    """M (f32, [k, j]) with forward(x) == x @ M + b."""
    I = np.eye(N, dtype=np.float64)
    M = _np_forward(I, perm_logit, abcd, np.zeros((N,), np.float64))
    return M.astype(np.float32)


# ---------------------------------------------------------------------------
# Device kernel
# ---------------------------------------------------------------------------

_BUILT = {}


def _build_nc(mode):
    import concourse.bacc as bacc
    import concourse.mybir as mybir
    from concourse.tile import TileContext

    f32 = mybir.dt.float32
    f32r = mybir.dt.float32r
    bf16 = mybir.dt.bfloat16
    io_dt = bf16 if mode == "bf16" else f32r
    x_dram_dt = f32 if mode == "bf16" else f32r

    nc = bacc.Bacc(None, target_bir_lowering=False)

    # xq[g, p, kt, c] = x[g*512 + c, kt*128 + p]  (pre-transposed on host)
    x_d = nc.dram_tensor(
        "xq", [N_GROUPS, 128, N_KT, GCOLS], x_dram_dt, kind="ExternalInput"
    )
    m_d = nc.dram_tensor("mmat", [128, N_KT, N], io_dt, kind="ExternalInput")
    b_d = nc.dram_tensor("bias", [128, N], f32, kind="ExternalInput")
    o_d = nc.dram_tensor("out", [B_CORE, N], f32, kind="ExternalOutput")

    with TileContext(nc) as tc:
        with (
            tc.tile_pool(name="const", bufs=1) as const,
            tc.tile_pool(name="osb", bufs=6) as out_pool,
            tc.tile_pool(name="ops", bufs=8, space="PSUM") as psum_pool,
        ):
            m_sb = const.tile([128, N_KT, N], io_dt)
            xg_sb = const.tile([128, N_GROUPS, N_KT, GCOLS], io_dt)
            bias_sb = const.tile([128, N], f32)
            xs_sb = None
            if mode == "bf16":
                # f32 staging for x ahead of the ACT cast
                xs_sb = const.tile([128, N_GROUPS, N_KT, GCOLS], f32)

            def x_dst(g, kt):
                t = xs_sb if mode == "bf16" else xg_sb
                if kt is None:
                    return t[:, g, :, :]
                return t[:, g, kt, :]

            def cast_x(g, kt):
                # ACT engine is otherwise idle: it only runs these casts,
                # so chunk-order emission can never head-of-line block.
                if mode == "bf16":
                    nc.scalar.copy(xg_sb[:, g, kt, :], xs_sb[:, g, kt, :])

            # ---- HAM warmup ----
            # The PE clock-gate (HAM) only unthrottles 1.2->2.4GHz after
            # ~3.4us of sustained PE activity; without this, the first
            # ~10 real matmuls run at ~630ns instead of 379ns.  Burn the
            # 6-12us DMA-wait window on dummy matmuls over memset data so
            # the real stream starts already warm.  po_warm is never read:
            # its PSUM bank is reclaimed by the ramp (start=True clears).
            warm_sb = const.tile([128, 640], io_dt)
            nc.gpsimd.memset(warm_sb[:], 0.0)
            po_warm = psum_pool.tile([128, 512], f32, name="po", tag="po")
            for _ in range(12):
                nc.tensor.matmul(
                    po_warm[:], warm_sb[:, 0:128], warm_sb[:, 128:640],
                    start=True, stop=True,
                )

            # ---- load schedule ----
            # sync ring FIFO: per-kt (M, x-group0) pairs feed the ramp
            # (kt0's M split per-jc so the first matmul issues sooner),
            # then bias and group 1 per-kt; stores follow in FIFO order.
            # Groups 2/3 are triggered from the ACT queue *after* the
            # group-0 casts, so their 2MB bulk transfers can neither
            # delay the ramp nor make the stores queue behind them.
            nc.sync.dma_start(m_sb[:, 0, 0:512], m_d[:, 0, 0:512])
            nc.sync.dma_start(x_dst(0, 0), x_d[0, :, 0, :])
            nc.sync.dma_start(m_sb[:, 0, 512:1024], m_d[:, 0, 512:1024])
            if mode == "bf16":
                # split the first cast so btile 0's weights are ready asap
                nc.scalar.copy(xg_sb[:, 0, 0, 0:128], xs_sb[:, 0, 0, 0:128])
                nc.scalar.copy(xg_sb[:, 0, 0, 128:512], xs_sb[:, 0, 0, 128:512])
            for kt in range(1, N_KT):
                nc.sync.dma_start(m_sb[:, kt, :], m_d[:, kt, :])
                nc.sync.dma_start(x_dst(0, kt), x_d[0, :, kt, :])
                cast_x(0, kt)
            nc.sync.dma_start(bias_sb[:], b_d[:])
            for g in (1, 2, 3):
                for kt in range(N_KT):
                    nc.sync.dma_start(x_dst(g, kt), x_d[g, :, kt, :])
                    cast_x(g, kt)

            # ---- compute ----
            def mm_btile(g, bt, po, kt):
                lhsT = xg_sb[:, g, kt, bt * 128 : (bt + 1) * 128]
                for jc in range(2):
                    nc.tensor.matmul(
                        po[jc][:],
                        lhsT,
                        m_sb[:, kt, jc * 512 : (jc + 1) * 512],
                        start=(kt == 0),
                        stop=(kt == N_KT - 1),
                    )

            def new_po():
                return [
                    psum_pool.tile([128, 512], f32, name="po", tag="po")
                    for _ in range(2)
                ]

            def evict(t, po, ring=None):
                # per-jc DVE bias-add + 256KB store: the first half leaves
                # while the second is still accumulating.  The last btiles
                # store via the ACT ring (idle after the casts) so their
                # ~0.6us store triggers don't serialize on the sync queue
                # behind the earlier btiles' stores.
                ring = ring or nc.sync
                out_sb = out_pool.tile([128, N], f32, name="out_sb", tag="out_sb")
                for jc in range(2):
                    lo, hi = jc * 512, (jc + 1) * 512
                    nc.vector.tensor_add(
                        out_sb[:, lo:hi], po[jc][:], bias_sb[:, lo:hi]
                    )
                    ring.dma_start(
                        o_d[t * 128 : (t + 1) * 128, lo:hi], out_sb[:, lo:hi]
                    )

            # Ramp: group 0's 4 btiles kt-major (all 8 PSUM banks); each
            # arriving (M, x) chunk pair feeds 8 matmuls.
            po_r = [new_po() for _ in range(4)]
            for kt in range(N_KT):
                for bt in range(4):
                    mm_btile(0, bt, po_r[bt], kt)
            for bt in range(4):
                evict(bt, po_r[bt])

            # Steady state: one btile at a time, PSUM pool rotation keeps
            # 4 btiles of slack between accumulate and eviction.
            for t in range(4, N_BT):
                g, bt = divmod(t, 4)
                po = new_po()
                for kt in range(N_KT):
                    mm_btile(g, bt, po, kt)
                evict(t, po, ring=nc.scalar if t >= N_BT - 2 else None)

    nc.compile()
    return nc


def _get_nc(mode):
    if mode not in _BUILT:
        _BUILT[mode] = _build_nc(mode)
    return _BUILT[mode]


LAST_RUN = {}


def _install_axon_ntff_shim():
    """Provide the missing ``antenv.axon_hooks`` module so
    ``run_bass_kernel_spmd(trace=True)`` can capture NTFF profiles under
    axon.  The hook drives ``axon_{start,stop}_nrt_profile`` in
    libaxon_pjrt.so directly (same ABI trn_boot uses)."""
    import contextlib
    import ctypes
    import sys
    import types

    if "antenv.axon_hooks" in sys.modules:
        return
    so_path = "/opt/axon/libaxon_pjrt.so"
    lib = ctypes.CDLL(so_path)
    if not hasattr(lib, "axon_start_nrt_profile"):
        raise RuntimeError("libaxon_pjrt.so lacks axon_start_nrt_profile")
    lib.axon_start_nrt_profile.argtypes = [
        ctypes.POINTER(ctypes.c_int64),
        ctypes.c_size_t,
    ]
    lib.axon_start_nrt_profile.restype = ctypes.c_int64
    lib.axon_stop_nrt_profile.argtypes = [ctypes.c_char_p]
    lib.axon_stop_nrt_profile.restype = ctypes.c_int64

    @contextlib.contextmanager
    def _hook(output_dir, device_ids):
        import jax

        jax.devices()
        if device_ids:
            ids = (ctypes.c_int64 * len(device_ids))(*device_ids)
            rc = lib.axon_start_nrt_profile(ids, len(device_ids))
        else:
            rc = lib.axon_start_nrt_profile(None, 0)
        if rc != 0:
            raise RuntimeError(f"axon_start_nrt_profile rc={rc}")
        try:
            yield
        finally:
            n = lib.axon_stop_nrt_profile(str(output_dir).encode())
            print(f"ntff profile: {n} file(s) written to {output_dir}")

    mod = types.ModuleType("antenv.axon_hooks")
    mod.get_axon_ntff_profile_hook = lambda: _hook
    mod.set_axon_ntff_profile_hook = lambda h: None
    sys.modules["antenv.axon_hooks"] = mod
    import antenv

    antenv.axon_hooks = mod


def kernel(x, perm_logit, abcd, b, _trace=False):
    import ml_dtypes
    import concourse.bass_utils as bass_utils
    from concourse.bass_utils import run_bass_kernel_spmd

    if _trace:
        try:
            _install_axon_ntff_shim()
            # artifact upload needs a remote bucket; stub it for local runs
            bass_utils.upload_artifacts = lambda tmpdir: tmpdir
        except Exception as e:  # degrade to untraced run
            print("trace setup failed:", e)
            _trace = False

    x = np.ascontiguousarray(np.asarray(x, np.float32))
    M = _build_matrix(perm_logit, abcd)  # [k, j] f32

    m_in = np.ascontiguousarray(M.reshape(N_KT, 128, N).transpose(1, 0, 2))
    if MM_MODE == "bf16":
        m_in = m_in.astype(ml_dtypes.bfloat16)
    bias_in = np.ascontiguousarray(
        np.broadcast_to(np.asarray(b, np.float32), (128, N))
    )

    nc = _get_nc(MM_MODE)
    in_maps = []
    for c in range(N_CORES):
        xc = x[c * B_CORE : (c + 1) * B_CORE]  # [2048, 1024]
        # xq[g, p, kt, c] = xc[g*512 + c, kt*128 + p]
        xq = np.ascontiguousarray(
            xc.T.reshape(N_KT, 128, N_GROUPS, GCOLS).transpose(2, 1, 0, 3)
        )
        in_maps.append({"xq": xq, "mmat": m_in, "bias": bias_in})
    res = run_bass_kernel_spmd(
        nc, in_maps, core_ids=list(range(N_CORES)), trace=_trace
    )
    LAST_RUN["results"] = res
    LAST_RUN["exec_time_ns"] = res.exec_time_ns
    out = np.concatenate([r["out"] for r in res.results], axis=0)
    return out


# revision 24
# speedup vs baseline: 1.0046x; 1.0046x over previous
"""Butterfly (nn_Butterfly) forward as a single dense matmul on 8 TRN2 cores.

The reference butterfly network is linear in x: h starts as (x, 0) complex
pairs, every perm/diag factor is a real-linear map with coefficients that
depend only on (perm_logit, abcd), and the output takes the real part and
adds b.  So forward(x) == x @ M + b where M = forward(I_1024) with b=0.
M is built on the host from the ~16KB params (cheap, exact), then the
device kernel is a data-parallel [2048,1024] @ [1024,1024] matmul per core.

Structure (v3):
  - x is pre-transposed on the host during sharding, so the contraction
    dim lands on SBUF partitions straight off the DMA (no PE transposes).
  - bf16 operands: measured on HW, bf16 matmuls stream at the 215ns/MM
    roofline with 97ns (FWL) weight loads, while f32r runs ~15% slower
    with 200ns weight loads.  x is DMA'd as f32 on the sync HWDGE ring
    into a staging tile and cast f32->bf16 by the otherwise-idle ACT
    engine (the SWDGE cast-on-DMA path starts ~5us late and trickles).
  - DMA schedule: M chunks + bias + stores on the sync (SP) ring,
    x chunks on the scalar (ACT) ring.  The two rings drain concurrently
    (SDMA round-robins per packet), so each ramp kt's (M, x) chunk pair
    lands in ~1.4us < the 1.73us of matmuls it feeds - the PE never
    starves after the first chunk.  kt0's chunks are split finer so the
    first matmul issues as early as possible.
  - PSUM: 8 accumulator banks; ramp runs 4 btiles kt-major against the
    arriving chunks, steady state runs one btile at a time.
  - stores are per-512-column half, so the tail after the last matmul is
    one DVE add + one 256KB store.
"""

import numpy as np

N = 1024
B_FULL = 16384
N_CORES = 8
B_CORE = B_FULL // N_CORES  # 2048
N_KT = N // 128  # 8 contraction tiles
N_GROUPS = 4  # x column groups per core
GCOLS = B_CORE // N_GROUPS  # 512 batch columns per group
N_BT = B_CORE // 128  # 16 output row tiles

# "bf16": bf16 matmuls, x cast on ACT (fastest measured)
# "f32r": both operands f32r (exact fp32; ~15% slower PE stream on HW)
MM_MODE = "bf16"


# ---------------------------------------------------------------------------
# Host side: collapse the butterfly network to a single matrix
# ---------------------------------------------------------------------------

def _abcd_offsets(n):
    offs = []
    off = 0
    m = n
    while m >= 2:
        offs.append((m, off))
        off += 2 * m
        m //= 2
    return offs, off


def _np_forward(x, perm_logit, abcd, b):
    """Float64 numpy port of reference._forward (op-for-op)."""
    x = np.asarray(x, np.float64)
    perm_logit = np.asarray(perm_logit, np.float64)
    abcd = np.asarray(abcd, np.float64)
    b = np.asarray(b, np.float64)
    n = x.shape[-1]
    Bn = x.shape[0]
    offs, _ = _abcd_offsets(n)
    h = np.stack([x, np.zeros_like(x)], axis=-1)
    perm_sizes = [m for (m, _) in offs if m >= 4]
    for d in range(perm_logit.shape[0]):
        p = 1.0 / (1.0 + np.exp(-perm_logit[d]))
        for m in reversed(perm_sizes):
            h = h.reshape(Bn, n // m, m, 2)
            eo = np.concatenate([h[:, :, 0::2], h[:, :, 1::2]], axis=2)
            h = (1 - p[0]) * h + p[0] * eo
            h1, h2 = h[:, :, : m // 2], h[:, :, m // 2 :]
            h1 = (1 - p[1]) * h1 + p[1] * h1[:, :, ::-1]
            h2 = (1 - p[2]) * h2 + p[2] * h2[:, :, ::-1]
            h = np.concatenate([h1, h2], axis=2).reshape(Bn, n, 2)
        for (m, off) in reversed(offs):
            ABCD = abcd[d, off : off + 2 * m].reshape(2, 2, m // 2, 2)
            hv = h.reshape(Bn, n // m, 2, m // 2, 2)
            xr, xi = hv[..., 0], hv[..., 1]
            Ar, Ai = ABCD[..., 0], ABCD[..., 1]
            yr = np.einsum("ijk,bnjk->bnik", Ar, xr) - np.einsum(
                "ijk,bnjk->bnik", Ai, xi
            )
            yi = np.einsum("ijk,bnjk->bnik", Ar, xi) + np.einsum(
                "ijk,bnjk->bnik", Ai, xr
            )
            h = np.stack([yr, yi], axis=-1).reshape(Bn, n, 2)
    return b + h[..., 0]


def _build_matrix(perm_logit, abcd):
    """M (f32, [k, j]) with forward(x) == x @ M + b."""
    I = np.eye(N, dtype=np.float64)
    M = _np_forward(I, perm_logit, abcd, np.zeros((N,), np.float64))
    return M.astype(np.float32)


# ---------------------------------------------------------------------------
# Device kernel
# ---------------------------------------------------------------------------

_BUILT = {}


def _build_nc(mode):
    import concourse.bacc as bacc
    import concourse.mybir as mybir
    from concourse.tile import TileContext

    f32 = mybir.dt.float32
    f32r = mybir.dt.float32r
    bf16 = mybir.dt.bfloat16
    io_dt = bf16 if mode == "bf16" else f32r
    x_dram_dt = f32 if mode == "bf16" else f32r

    nc = bacc.Bacc(None, target_bir_lowering=False)

    # xq[g, p, kt, c] = x[g*512 + c, kt*128 + p]  (pre-transposed on host)
    x_d = nc.dram_tensor(
        "xq", [N_GROUPS, 128, N_KT, GCOLS], x_dram_dt, kind="ExternalInput"
    )
    m_d = nc.dram_tensor("mmat", [128, N_KT, N], io_dt, kind="ExternalInput")
    b_d = nc.dram_tensor("bias", [128, N], f32, kind="ExternalInput")
    o_d = nc.dram_tensor("out", [B_CORE, N], f32, kind="ExternalOutput")

    with TileContext(nc) as tc:
        with (
            tc.tile_pool(name="const", bufs=1) as const,
            tc.tile_pool(name="osb", bufs=6) as out_pool,
            tc.tile_pool(name="ops", bufs=8, space="PSUM") as psum_pool,
        ):
            m_sb = const.tile([128, N_KT, N], io_dt)
            xg_sb = const.tile([128, N_GROUPS, N_KT, GCOLS], io_dt)
            bias_sb = const.tile([128, N], f32)
            xs_sb = None
            if mode == "bf16":
                # f32 staging for x ahead of the ACT cast
                xs_sb = const.tile([128, N_GROUPS, N_KT, GCOLS], f32)

            def x_dst(g, kt):
                t = xs_sb if mode == "bf16" else xg_sb
                if kt is None:
                    return t[:, g, :, :]
                return t[:, g, kt, :]

            def cast_x(g, kt):
                # ACT engine is otherwise idle: it only runs these casts,
                # so chunk-order emission can never head-of-line block.
                if mode == "bf16":
                    nc.scalar.copy(xg_sb[:, g, kt, :], xs_sb[:, g, kt, :])

            # ---- HAM warmup ----
            # The PE clock-gate (HAM) only unthrottles 1.2->2.4GHz after
            # ~3.4us of sustained PE activity; without this, the first
            # ~10 real matmuls run at ~630ns instead of 379ns.  Burn the
            # 6-12us DMA-wait window on dummy matmuls over memset data so
            # the real stream starts already warm.  po_warm is never read:
            # its PSUM bank is reclaimed by the ramp (start=True clears).
            warm_sb = const.tile([128, 640], io_dt)
            nc.gpsimd.memset(warm_sb[:], 0.0)
            po_warm = psum_pool.tile([128, 512], f32, name="po", tag="po")
            for _ in range(8):
                nc.tensor.matmul(
                    po_warm[:], warm_sb[:, 0:128], warm_sb[:, 128:640],
                    start=True, stop=True,
                )

            # ---- load schedule ----
            # sync ring FIFO: per-kt (M, x-group0) pairs feed the ramp
            # (kt0's M split per-jc so the first matmul issues sooner),
            # then bias and group 1 per-kt; stores follow in FIFO order.
            # Groups 2/3 are triggered from the ACT queue *after* the
            # group-0 casts, so their 2MB bulk transfers can neither
            # delay the ramp nor make the stores queue behind them.
            nc.sync.dma_start(m_sb[:, 0, 0:512], m_d[:, 0, 0:512])
            nc.sync.dma_start(x_dst(0, 0)[:, 0:256], x_d[0, :, 0, 0:256])
            nc.sync.dma_start(m_sb[:, 0, 512:1024], m_d[:, 0, 512:1024])
            nc.sync.dma_start(x_dst(0, 0)[:, 256:512], x_d[0, :, 0, 256:512])
            if mode == "bf16":
                # split the first cast so btile 0's weights are ready asap
                nc.scalar.copy(xg_sb[:, 0, 0, 0:128], xs_sb[:, 0, 0, 0:128])
                nc.scalar.copy(xg_sb[:, 0, 0, 128:256], xs_sb[:, 0, 0, 128:256])
                nc.scalar.copy(xg_sb[:, 0, 0, 256:512], xs_sb[:, 0, 0, 256:512])
            for kt in range(1, N_KT):
                nc.sync.dma_start(m_sb[:, kt, :], m_d[:, kt, :])
                nc.sync.dma_start(x_dst(0, kt), x_d[0, :, kt, :])
                cast_x(0, kt)
            nc.sync.dma_start(bias_sb[:], b_d[:])
            for g in (1, 2, 3):
                for kt in range(N_KT):
                    nc.sync.dma_start(x_dst(g, kt), x_d[g, :, kt, :])
                    cast_x(g, kt)

            # ---- compute ----
            def mm_btile(g, bt, po, kt):
                lhsT = xg_sb[:, g, kt, bt * 128 : (bt + 1) * 128]
                for jc in range(2):
                    nc.tensor.matmul(
                        po[jc][:],
                        lhsT,
                        m_sb[:, kt, jc * 512 : (jc + 1) * 512],
                        start=(kt == 0),
                        stop=(kt == N_KT - 1),
                    )

            def new_po():
                return [
                    psum_pool.tile([128, 512], f32, name="po", tag="po")
                    for _ in range(2)
                ]

            def evict(t, po, ring=None):
                # per-jc DVE bias-add + 256KB store: the first half leaves
                # while the second is still accumulating.  The last btiles
                # store via the ACT ring (idle after the casts) so their
                # ~0.6us store triggers don't serialize on the sync queue
                # behind the earlier btiles' stores.
                ring = ring or nc.sync
                out_sb = out_pool.tile([128, N], f32, name="out_sb", tag="out_sb")
                for jc in range(2):
                    lo, hi = jc * 512, (jc + 1) * 512
                    nc.vector.tensor_add(
                        out_sb[:, lo:hi], po[jc][:], bias_sb[:, lo:hi]
                    )
                    ring.dma_start(
                        o_d[t * 128 : (t + 1) * 128, lo:hi], out_sb[:, lo:hi]
                    )

            # Ramp: group 0's 4 btiles kt-major (all 8 PSUM banks); each
            # arriving (M, x) chunk pair feeds 8 matmuls.
            po_r = [new_po() for _ in range(4)]
            for kt in range(N_KT):
                for bt in range(4):
                    mm_btile(0, bt, po_r[bt], kt)
            for bt in range(4):
                evict(bt, po_r[bt])

            # Steady state: one btile at a time, PSUM pool rotation keeps
            # 4 btiles of slack between accumulate and eviction.
            for t in range(4, N_BT):
                g, bt = divmod(t, 4)
                po = new_po()
                for kt in range(N_KT):
                    mm_btile(g, bt, po, kt)
                evict(t, po, ring=nc.scalar if t >= N_BT - 2 else None)

    nc.compile()
    return nc


def _get_nc(mode):
    if mode not in _BUILT:
        _BUILT[mode] = _build_nc(mode)
    return _BUILT[mode]


LAST_RUN = {}


def _install_axon_ntff_shim():
    """Provide the missing ``antenv.axon_hooks`` module so
    ``run_bass_kernel_spmd(trace=True)`` can capture NTFF profiles under
    axon.  The hook drives ``axon_{start,stop}_nrt_profile`` in
    libaxon_pjrt.so directly (same ABI trn_boot uses)."""
    import contextlib
    import ctypes
    import sys
    import types

    if "antenv.axon_hooks" in sys.modules:
        return
    so_path = "/opt/axon/libaxon_pjrt.so"
    lib = ctypes.CDLL(so_path)
    if not hasattr(lib, "axon_start_nrt_profile"):
        raise RuntimeError("libaxon_pjrt.so lacks axon_start_nrt_profile")
    lib.axon_start_nrt_profile.argtypes = [
        ctypes.POINTER(ctypes.c_int64),
        ctypes.c_size_t,
    ]
    lib.axon_start_nrt_profile.restype = ctypes.c_int64
    lib.axon_stop_nrt_profile.argtypes = [ctypes.c_char_p]
    lib.axon_stop_nrt_profile.restype = ctypes.c_int64

    @contextlib.contextmanager
    def _hook(output_dir, device_ids):
        import jax

        jax.devices()
        if device_ids:
            ids = (ctypes.c_int64 * len(device_ids))(*device_ids)
            rc = lib.axon_start_nrt_profile(ids, len(device_ids))
        else:
            rc = lib.axon_start_nrt_profile(None, 0)
        if rc != 0:
            raise RuntimeError(f"axon_start_nrt_profile rc={rc}")
        try:
            yield
        finally:
            n = lib.axon_stop_nrt_profile(str(output_dir).encode())
            print(f"ntff profile: {n} file(s) written to {output_dir}")

    mod = types.ModuleType("antenv.axon_hooks")
    mod.get_axon_ntff_profile_hook = lambda: _hook
    mod.set_axon_ntff_profile_hook = lambda h: None
    sys.modules["antenv.axon_hooks"] = mod
    import antenv

    antenv.axon_hooks = mod


def kernel(x, perm_logit, abcd, b, _trace=False):
    import ml_dtypes
    import concourse.bass_utils as bass_utils
    from concourse.bass_utils import run_bass_kernel_spmd

    if _trace:
        try:
            _install_axon_ntff_shim()
            # artifact upload needs a remote bucket; stub it for local runs
            bass_utils.upload_artifacts = lambda tmpdir: tmpdir
        except Exception as e:  # degrade to untraced run
            print("trace setup failed:", e)
            _trace = False

    x = np.ascontiguousarray(np.asarray(x, np.float32))
    M = _build_matrix(perm_logit, abcd)  # [k, j] f32

    m_in = np.ascontiguousarray(M.reshape(N_KT, 128, N).transpose(1, 0, 2))
    if MM_MODE == "bf16":
        m_in = m_in.astype(ml_dtypes.bfloat16)
    bias_in = np.ascontiguousarray(
        np.broadcast_to(np.asarray(b, np.float32), (128, N))
    )

    nc = _get_nc(MM_MODE)
    in_maps = []
    for c in range(N_CORES):
        xc = x[c * B_CORE : (c + 1) * B_CORE]  # [2048, 1024]
        # xq[g, p, kt, c] = xc[g*512 + c, kt*128 + p]
        xq = np.ascontiguousarray(
            xc.T.reshape(N_KT, 128, N_GROUPS, GCOLS).transpose(2, 1, 0, 3)
        )
        in_maps.append({"xq": xq, "mmat": m_in, "bias": bias_in})
    res = run_bass_kernel_spmd(
        nc, in_maps, core_ids=list(range(N_CORES)), trace=_trace
    )
    LAST_RUN["results"] = res
    LAST_RUN["exec_time_ns"] = res.exec_time_ns
    out = np.concatenate([r["out"] for r in res.results], axis=0)
    return out
